# revision 4
# baseline (speedup 1.0000x reference)
"""Multi-head self-attention (B=16, N=784, D=768, H=12) on 8 trn2 cores.

Strategy: pure data-parallel over batch (2 batches per core, no collectives).
Per batch, on-device:
  A) X [784,768] is PE-transposed into XT [768,784] (bf16).
  B) QKV projection. Q,K are produced in transposed layout QKT [f, t]
     (stationary = Wqkv column block, moving = XT); V is produced in natural
     layout [t, f] (stationary = XT chunk, moving = Wqkv V columns) and packed
     into an augmented slab [t, 12*(64+1)] whose per-head 65th column is 1.0.
  C) Per head: scores^T [tj, ti] = K^T-chunk.T @ Q^T (K=64 contraction);
     softmax without max-subtraction (scores are O(1) here): exp on ACT with
     the 1/8 scale fused in; PV with the ones-augmented V slab gives
     O^T[64,ti] plus the softmax denominator in row 64 for free; normalize
     with a reciprocal + partition-broadcast multiply during PSUM->SBUF copy.
  D) Output projection from OT (already the required lhsT layout) + bias via
     rank-1 accumulating matmuls.
All matmuls run in bf16 (PSUM accumulates fp32).
"""

from contextlib import ExitStack

import numpy as np

import concourse.bass as bass
import concourse.mybir as mybir
import concourse.tile as tile
from concourse import bacc
from concourse.bass_utils import run_bass_kernel_spmd
from concourse.masks import make_identity

dt = mybir.dt
AF = mybir.ActivationFunctionType

B, N, D = 16, 784, 768
H, HD = 12, 64
F3 = 3 * D  # 2304
N_CORES = 8
BPC = B // N_CORES  # batches per core

# token chunks: 784 = 6*128 + 16
T_CHUNKS = [(i * 128, min(128, N - i * 128)) for i in range((N + 127) // 128)]
NT = len(T_CHUNKS)  # 7
ND = D // 128  # 6 d-chunks
# psum column groups (bank-aligned: one fp32 bank holds 512)
COLS_N = [(0, 512), (512, N - 512)]   # over 784 tokens
COLS_D = [(0, 512), (512, D - 512)]   # over 768 features


def _setup_consts(nc, P, cdt, aps):
    ident = P["konst"].tile([128, 128], dt.float32, name="ident")
    make_identity(nc, ident[:])
    ones_c = P["konst"].tile([1, N], cdt, name="ones_c")
    nc.vector.memset(ones_c[:], 1.0)

    bqc = P["konst"].tile([128, F3 // 128], dt.float32, name="bqc")
    nc.sync.dma_start(bqc[:], aps["bqc"][:])
    bstage = P["konst"].tile([1, D], dt.float32, name="bstage")
    nc.sync.dma_start(bstage[:], aps["bqv"][:])
    bqv16 = P["konst"].tile([1, D], cdt, name="bqv16")
    nc.vector.tensor_copy(bqv16[:], bstage[:])
    bstage2 = P["konst"].tile([1, D], dt.float32, name="bstage2")
    nc.sync.dma_start(bstage2[:], aps["bo"][:])
    bo16 = P["konst"].tile([1, D], cdt, name="bo16")
    nc.vector.tensor_copy(bo16[:], bstage2[:])

    wq16, wo16 = [], []
    for di in range(ND):
        st = P["stage"].tile([128, F3], dt.float32, name="wst", tag="wst")
        nc.sync.dma_start(st[:], aps["wqkv"][di * 128:(di + 1) * 128, :])
        w = P["wq"].tile([128, F3], cdt, name=f"wq{di}", tag="wq")
        nc.vector.tensor_copy(w[:], st[:])
        wq16.append(w)
    for di in range(ND):
        st = P["stage"].tile([128, F3], dt.float32, name="wost", tag="wst")
        nc.sync.dma_start(st[:, 0:D], aps["wo"][di * 128:(di + 1) * 128, :])
        w = P["wo"].tile([128, D], cdt, name=f"wo{di}", tag="wo")
        nc.vector.tensor_copy(w[:], st[:, 0:D])
        wo16.append(w)
    return dict(ident=ident, ones_c=ones_c, bqc=bqc, bqv16=bqv16, bo16=bo16,
                wq16=wq16, wo16=wo16)


def _phase_a(nc, P, C, cdt, aps, b):
    """Load X and transpose to XT [768, 784] bf16."""
    xt16 = [P["xt"].tile([128, N], cdt, name=f"xt{b}_{di}", tag="xt")
            for di in range(ND)]
    for t0, p in T_CHUNKS:
        x32 = P["xin"].tile([128, D], dt.float32, name="x32", tag="x32")
        nc.sync.dma_start(x32[0:p, :], aps["xs"][b, t0:t0 + p, :])
        for di in range(ND):
            tr = P["ps_tr"].tile([128, 128], dt.float32, name="tr", tag="tr")
            nc.tensor.transpose(tr[0:128, 0:p],
                                x32[0:p, di * 128:(di + 1) * 128],
                                C["ident"][0:p, 0:p])
            nc.vector.tensor_copy(xt16[di][:, t0:t0 + p], tr[0:128, 0:p])
    return xt16


def _phase_b_qk(nc, P, C, cdt, b, xt16):
    """Q,K in transposed layout: 12 tiles [128, 784]."""
    qkt16 = []
    for fi in range(12):  # Q: 0..5, K: 6..11
        qk_ps = P["ps_big"].tile([128, N], dt.float32, name="qk_ps", tag="big")
        for (c0, cw) in COLS_N:
            for di in range(ND):
                nc.tensor.matmul(
                    qk_ps[:, c0:c0 + cw],
                    C["wq16"][di][:, fi * 128:(fi + 1) * 128],
                    xt16[di][:, c0:c0 + cw],
                    start=(di == 0), stop=(di == ND - 1))
        q = P["qkt"].tile([128, N], cdt, name=f"qkt{b}_{fi}", tag="qkt")
        nc.vector.tensor_scalar_add(q[:], qk_ps[:], C["bqc"][0:128, fi:fi + 1])
        qkt16.append(q)
    return qkt16


def _phase_b_v(nc, P, C, cdt, b, xt16):
    """V natural layout, packed per head with a trailing ones column."""
    vt16 = []
    for (t0, p), ti in zip(T_CHUNKS, range(NT)):
        v_ps = P["ps_big"].tile([128, N], dt.float32, name="v_ps", tag="big")
        for (c0, cw) in COLS_D:
            for di in range(ND):
                nc.tensor.matmul(
                    v_ps[0:p, c0:c0 + cw],
                    xt16[di][:, t0:t0 + p],
                    C["wq16"][di][:, 2 * D + c0:2 * D + c0 + cw],
                    start=(di == 0), stop=False)
            nc.tensor.matmul(
                v_ps[0:p, c0:c0 + cw],
                C["ones_c"][0:1, t0:t0 + p],
                C["bqv16"][0:1, c0:c0 + cw],
                start=False, stop=True)
        vt = P["vt"].tile([128, H, HD + 1], cdt, name=f"vt{b}_{ti}", tag="vt")
        nc.vector.tensor_copy(vt[0:p, :, 0:HD],
                              v_ps[0:p, 0:D].rearrange("p (h d) -> p h d", h=H))
        nc.vector.memset(vt[0:p, :, HD:HD + 1], 1.0)
        vt16.append(vt)
    return vt16


def _head_scores(nc, P, cdt, qt, kt, ro):
    """scores^T -> exp, returns 7 expS^T tiles [tj, 784] bf16."""
    exl = []
    for (t0, pj), tj in zip(T_CHUNKS, range(NT)):
        sc_ps = P["ps_big"].tile([128, N], dt.float32, name="sc_ps", tag="big")
        for (c0, cw) in COLS_N:
            nc.tensor.matmul(
                sc_ps[0:pj, c0:c0 + cw],
                kt[ro:ro + HD, t0:t0 + pj],
                qt[ro:ro + HD, c0:c0 + cw],
                start=True, stop=True)
        ex = P["ex"].tile([128, N], cdt, name="ex", tag="ex")
        nc.scalar.activation(ex[0:pj, :], sc_ps[0:pj, :], AF.Exp,
                             scale=float(HD) ** -0.5)
        exl.append(ex)
    return exl


def _head_pv(nc, P, h, vt16, exl):
    """PV with ones-augmented V: psum [65, 784]; row 64 = softmax denom."""
    ot_ps = P["ps_big"].tile([HD + 1, N], dt.float32, name="ot_ps", tag="big")
    for (c0, cw) in COLS_N:
        for (t0, pj), tj in zip(T_CHUNKS, range(NT)):
            nc.tensor.matmul(
                ot_ps[0:HD + 1, c0:c0 + cw],
                vt16[tj][0:pj, h, 0:HD + 1],
                exl[tj][0:pj, c0:c0 + cw],
                start=(tj == 0), stop=(tj == NT - 1))
    return ot_ps


def _phase_c(nc, P, cdt, b, qkt16, vt16):
    ot16 = [P["ot"].tile([128, N], cdt, name=f"ot{b}_{oi}", tag="ot")
            for oi in range(ND)]
    for h in range(H):
        qt, kt, ro = qkt16[h // 2], qkt16[6 + h // 2], (h % 2) * HD
        exl = _head_scores(nc, P, cdt, qt, kt, ro)
        ot_ps = _head_pv(nc, P, h, vt16, exl)
        rec = P["recp"].tile([1, N], dt.float32, name="rec", tag="rec")
        nc.vector.reciprocal(rec[0:1, :], ot_ps[HD:HD + 1, :])
        brec = P["brec"].tile([HD, N], dt.float32, name="brec", tag="brec")
        nc.gpsimd.partition_broadcast(brec[0:HD, :], rec[0:1, :])
        nc.vector.tensor_mul(ot16[h // 2][ro:ro + HD, :],
                             ot_ps[0:HD, :], brec[0:HD, :])
    return ot16


def _phase_d(nc, P, C, aps, b, ot16):
    for (t0, p), ti in zip(T_CHUNKS, range(NT)):
        y_ps = P["ps_big"].tile([128, N], dt.float32, name="y_ps", tag="big")
        for (c0, cw) in COLS_D:
            for oi in range(ND):
                nc.tensor.matmul(
                    y_ps[0:p, c0:c0 + cw],
                    ot16[oi][:, t0:t0 + p],
                    C["wo16"][oi][:, c0:c0 + cw],
                    start=(oi == 0), stop=False)
            nc.tensor.matmul(
                y_ps[0:p, c0:c0 + cw],
                C["ones_c"][0:1, t0:t0 + p],
                C["bo16"][0:1, c0:c0 + cw],
                start=False, stop=True)
        y32 = P["yout"].tile([128, D], dt.float32, name="y32", tag="y32")
        nc.vector.tensor_copy(y32[0:p, :], y_ps[0:p, 0:D])
        nc.sync.dma_start(aps["ys"][b, t0:t0 + p, :], y32[0:p, :])


POOL_SPECS = [
    ("konst", 1, "SBUF"), ("stage", 2, "SBUF"), ("wq", ND, "SBUF"),
    ("wo", ND, "SBUF"), ("xin", 3, "SBUF"), ("xt", ND, "SBUF"),
    ("qkt", 12, "SBUF"), ("vt", NT, "SBUF"), ("ex", 9, "SBUF"),
    ("ot", ND, "SBUF"), ("brec", 2, "SBUF"), ("recp", 2, "SBUF"),
    ("yout", 3, "SBUF"),
    ("ps_tr", 2, "PSUM"), ("ps_big", 3, "PSUM"),
]


def build(compute_dt=dt.bfloat16):
    cdt = compute_dt
    nc = bacc.Bacc("TRN2", target_bir_lowering=False, debug=False)

    aps = {
        "xs": nc.dram_tensor("xs", [BPC, N, D], dt.float32, kind="ExternalInput").ap(),
        "wqkv": nc.dram_tensor("wqkv", [D, F3], dt.float32, kind="ExternalInput").ap(),
        "bqc": nc.dram_tensor("bqc", [128, F3 // 128], dt.float32, kind="ExternalInput").ap(),
        "bqv": nc.dram_tensor("bqv", [1, D], dt.float32, kind="ExternalInput").ap(),
        "wo": nc.dram_tensor("wo", [D, D], dt.float32, kind="ExternalInput").ap(),
        "bo": nc.dram_tensor("bo", [1, D], dt.float32, kind="ExternalInput").ap(),
        "ys": nc.dram_tensor("ys", [BPC, N, D], dt.float32, kind="ExternalOutput").ap(),
    }

    with ExitStack() as ctx:
        tc = ctx.enter_context(tile.TileContext(nc))
        P = {}
        for pname, bufs, space in POOL_SPECS:
            P[pname] = ctx.enter_context(
                tc.tile_pool(name=pname, bufs=bufs, space=space))

        C = _setup_consts(nc, P, cdt, aps)
        for b in range(BPC):
            xt16 = _phase_a(nc, P, C, cdt, aps, b)
            qkt16 = _phase_b_qk(nc, P, C, cdt, b, xt16)
            vt16 = _phase_b_v(nc, P, C, cdt, b, xt16)
            ot16 = _phase_c(nc, P, cdt, b, qkt16, vt16)
            _phase_d(nc, P, C, aps, b, ot16)

    nc.compile()
    return nc


_NC_CACHE = {}


def _get_nc():
    if "nc" not in _NC_CACHE:
        _NC_CACHE["nc"] = build()
    return _NC_CACHE["nc"]


def make_in_maps(x, Wqkv, bqkv, Wo, bo):
    x = np.ascontiguousarray(x, dtype=np.float32)
    Wqkv = np.ascontiguousarray(Wqkv, dtype=np.float32)
    bqkv = np.ascontiguousarray(bqkv, dtype=np.float32)
    Wo = np.ascontiguousarray(Wo, dtype=np.float32)
    bo = np.ascontiguousarray(bo, dtype=np.float32)
    bqc = np.ascontiguousarray(bqkv.reshape(F3 // 128, 128).T)
    bqv = np.ascontiguousarray(bqkv[2 * D:].reshape(1, D))
    bo_r = np.ascontiguousarray(bo.reshape(1, D))
    in_maps = []
    for c in range(N_CORES):
        in_maps.append({
            "xs": np.ascontiguousarray(x[c * BPC:(c + 1) * BPC]),
            "wqkv": Wqkv,
            "bqc": bqc,
            "bqv": bqv,
            "wo": Wo,
            "bo": bo_r,
        })
    return in_maps


def run(x, Wqkv, bqkv, Wo, bo, trace=False, **kw):
    nc = _get_nc()
    in_maps = make_in_maps(x, Wqkv, bqkv, Wo, bo)
    res = run_bass_kernel_spmd(nc, in_maps, list(range(N_CORES)), trace=trace, **kw)
    out = np.concatenate([res.results[c]["ys"] for c in range(N_CORES)], axis=0)
    return out, res


def kernel(x, Wqkv, bqkv, Wo, bo):
    out, _ = run(x, Wqkv, bqkv, Wo, bo)
    return out
